# revision 24
# baseline (speedup 1.0000x reference)
"""Trainium2 Bass kernel for CustomAttention (non-local block), fp8 edition.

Reference math (per batch b, one NeuronCore per batch):
    xf = x.reshape(C, N)                      # C=512, N=H*W=4096
    qT = Wq @ xf + bq                         # [64, N]   (d on partitions)
    kT = Wk @ xf + bk                         # [64, N]
    sT[j, i] = sum_d kT[d, j] * qT[d, i]      # scores, keys on partitions
    attn = softmax_j
    vT[n, e] = sum_c xf[c, n] Wv[e, c]        # v transposed: n on partitions
    out[e, i] = gamma * (sum_j (vT[j,e]+bv) a[i,j]) + x[e, i]
             = gamma * (sum_j vT[j,e] ẽ[j,i]) / Z[i] + (gamma*bv[e] + x[e,i])
    (softmax rows sum to 1, so bv contributes exactly gamma*bv per channel)

Numerics: all heavy matmuls run in fp8e4 (TRN e4m3, max ±240).
  - Wq/Wk/Wv are pre-scaled by 16 on the host (keeps them out of the fp8
    subnormal range); x / q / k / v / exp tiles are stored e4m3.
  - scores accumulate in fp32 PSUM as 256*s; exp applies scale=1/256 and
    a bias shift so exp outputs stay in [~2^-9, ~80] (margin to 240).
  - attn@v and the softmax row-sum Z use perf_mode=DoubleRow: two 128-row
    k-chunks contracted per matmul ([128, 2, M] stationary / [128, 2, N]
    moving 3D APs).
  - The residual add uses an exact fp32 copy of x.
"""

import numpy as np
import ml_dtypes
from contextlib import ExitStack

import concourse.mybir as mybir
import concourse.tile as tile
from concourse import bacc
from concourse.bass_utils import run_bass_kernel_spmd

B, C, HW, N, D = 8, 512, 64, 4096, 64
P = 128          # partitions
CB = C // P      # 4 channel chunks
JB = N // P      # 32 key chunks
JP = JB // 2     # 16 key chunk-pairs (DoubleRow)
IB = N // 512    # 8 query blocks
NB = 512         # query block width
F32 = mybir.dt.float32
F32R = mybir.dt.float32r
F8 = mybir.dt.float8e4
NP8 = ml_dtypes.float8_e4m3
DR = mybir.MatmulPerfMode.DoubleRow

WSCALE = 16.0         # host-side weight scale
EXPSCALE = 1.0 / (WSCALE * WSCALE)   # undo q,k scales inside exp
EXPBIAS = -5.6        # keeps exp outputs < ~100 (e4m3 max 240), rows alive

# exposed for test harness
LAST_RESULTS = None


def build_nc(gamma: float, bv_nonzero: bool):
    nc = bacc.Bacc(None, target_bir_lowering=False)

    # host-preformatted layouts (p = partition):
    #   x8   [p, (c n)] : x[c*128+p, n] as e4m3           (projections)
    #   xf   [C, N]     : exact fp32 x                     (residual)
    #   wqk8 [p, (c m)] : 16*concat(Wq,Wk)[m, c*128+p]     (m: 0-63 q, 64-127 k)
    #   wv8  [p, (c e)] : 16*Wv[e, c*128+p]
    #   bqk  [p, 1]     : 16*concat(bq, bk)
    #   gbv  [C, 1]     : gamma*bv
    x8 = nc.dram_tensor("x8", [P, CB * N], F8, kind="ExternalInput")
    xf = nc.dram_tensor("xf", [C, N], F32, kind="ExternalInput")
    wqk8 = nc.dram_tensor("wqk8", [P, CB * P], F8, kind="ExternalInput")
    wv8 = nc.dram_tensor("wv8", [P, CB * C], F8, kind="ExternalInput")
    bqk = nc.dram_tensor("bqk16", [P, 1], F32, kind="ExternalInput")
    gbv = nc.dram_tensor("gbv", [C, 1], F32, kind="ExternalInput")
    out = nc.dram_tensor("out", [C, N], F32, kind="ExternalOutput")

    x8_v = x8[:, :].rearrange("p (c n) -> p c n", c=CB)
    wqk_v = wqk8[:, :].rearrange("p (c m) -> p c m", c=CB)
    wv_v = wv8[:, :].rearrange("p (c e) -> p c e", c=CB)
    xf_v = xf[:, :].rearrange("(c p) n -> p c n", p=P)
    gbv_v = gbv[:, :].rearrange("(c p) n -> p c n", p=P)

    with tile.TileContext(nc) as tc, ExitStack() as stack:
        const = stack.enter_context(tc.tile_pool(name="const", bufs=1))
        big = stack.enter_context(tc.tile_pool(name="big", bufs=1))

        wqkT = const.tile([P, CB, P], F8, tag="wqkT")
        wvT = const.tile([P, CB, C], F8, tag="wvT")
        bqk_sb = const.tile([P, 1], F32, tag="bqk")
        expbias = const.tile([P, 1], F32, tag="expbias")
        ones8 = const.tile([P, 2, 32], F8, tag="ones8")
        gamma_col = const.tile([1, P], F32R, tag="gam")
        gbv_sb = const.tile([P, CB, 1], F32, tag="gbv")

        qT = big.tile([P, N], F8, tag="qT")      # rows 0-63 = q, 64-127 dup
        kT = big.tile([P, N], F8, tag="kT")      # rows 64-127 = k, 0-63 dup
        vT = big.tile([P, JB, C], F8, tag="vT")  # [j%128, j//128, e] = 16*v
        xsb = big.tile([P, CB, N], F32, tag="xsb")  # residual x, (e p) n
        xq8 = big.tile([P, CB, N], F8, tag="xq8")

        # ---------------- phase 0: loads + constants + projections ----------
        with (
            tc.tile_pool(name="ph0", bufs=1) as ph0,
            tc.tile_pool(name="ph0ps", bufs=1, space="PSUM") as ph0ps,
        ):
            t32 = ph0.tile([P, 2, 32], F32, tag="t32")
            nc.vector.memset(t32, 1.0)
            with nc.allow_low_precision(reason="exact 1.0 in fp8"):
                nc.vector.tensor_copy(ones8, t32)
            nc.vector.memset(expbias, EXPBIAS)
            g1 = ph0.tile([1, P], F32, tag="g1")
            nc.vector.memset(g1, 1.0 if gamma != 0.0 else 0.0)
            with nc.allow_low_precision(reason="f32r is 32-bit"):
                nc.vector.tensor_copy(gamma_col, g1)

            nc.scalar.dma_start(out=wqkT, in_=wqk_v)
            nc.scalar.dma_start(out=wvT, in_=wv_v)
            nc.scalar.dma_start(out=bqk_sb, in_=bqk[:, :])
            if bv_nonzero:
                nc.scalar.dma_start(out=gbv_sb, in_=gbv_v)

            # hoist every x8 chunk DMA ahead of the projection loop: descs
            # emitted between dependent ACTs would only issue after the
            # previous chunk's projections retire, starving the PE
            for nb in range(IB):
                ns = slice(nb * NB, (nb + 1) * NB)
                xq = nc.sync if nb % 2 == 0 else nc.scalar
                xq.dma_start(out=xq8[:, :, ns], in_=x8_v[:, :, ns])

            # q/k/v projections, one 512-wide n-block at a time
            for nb in range(IB):
                ns = slice(nb * NB, (nb + 1) * NB)

                psqk = ph0ps.tile([P, NB], F32, tag="qk", bufs=2)
                nc.tensor.matmul(psqk, wqkT[:, 0:2, :], xq8[:, 0:2, ns],
                                 start=True, stop=False, perf_mode=DR)
                nc.tensor.matmul(psqk, wqkT[:, 2:4, :], xq8[:, 2:4, ns],
                                 start=False, stop=True, perf_mode=DR)
                # engines are lane-locked: q rows live at psum 0-63, k at
                # 64-127; write each half, then DMA-duplicate the other half
                with nc.allow_low_precision(reason="fp8 attention pipeline"):
                    nc.scalar.activation(qT[0:D, ns], psqk[0:D, :],
                                         mybir.ActivationFunctionType.Identity,
                                         bias=bqk_sb[0:D, :])
                    nc.scalar.activation(kT[D:2 * D, ns], psqk[D:2 * D, :],
                                         mybir.ActivationFunctionType.Identity,
                                         bias=bqk_sb[D:2 * D, :])
                nc.gpsimd.dma_start(out=qT[D:2 * D, ns], in_=qT[0:D, ns])
                nc.gpsimd.dma_start(out=kT[0:D, ns], in_=kT[D:2 * D, ns])

                for sub in range(4):
                    jt = nb * 4 + sub
                    sl = slice(nb * NB + sub * P, nb * NB + (sub + 1) * P)
                    psv = ph0ps.tile([P, C], F32, tag="v", bufs=4)
                    nc.tensor.matmul(psv, xq8[:, 0:2, sl], wvT[:, 0:2, :],
                                     start=True, stop=False, perf_mode=DR)
                    nc.tensor.matmul(psv, xq8[:, 2:4, sl], wvT[:, 2:4, :],
                                     start=False, stop=True, perf_mode=DR)
                    with nc.allow_low_precision(reason="fp8 attention pipeline"):
                        nc.vector.tensor_copy(vT[:, jt, :], psv)

            # residual x rides the same queue AFTER the x8/weight loads so it
            # cannot starve the projections of HBM bandwidth; chunk ib is
            # needed only at the output combine of query-block ib (~25us+)
            for ib in range(IB):
                isl = slice(ib * NB, (ib + 1) * NB)
                xq = nc.sync if ib % 2 == 0 else nc.scalar
                xq.dma_start(out=xsb[:, :, isl], in_=xf_v[:, :, isl])
            if bv_nonzero:
                for e in range(CB):
                    nc.vector.tensor_scalar_add(
                        xsb[:, e, :], xsb[:, e, :], gbv_sb[:, e, :])

        # ---------------- main loop: attention ----------------
        with (
            tc.tile_pool(name="expp", bufs=8) as expp,
            tc.tile_pool(name="ost", bufs=8) as ost,
            tc.tile_pool(name="small", bufs=3) as small,
            tc.tile_pool(name="mps", bufs=1, space="PSUM") as mps,
        ):
            for ib in range(IB):
                isl = slice(ib * NB, (ib + 1) * NB)
                psum_z = mps.tile([32, NB], F32, tag="z")
                psum_pv = [mps.tile([P, NB], F32, tag=f"pv{e}", name=f"psum_pv{e}")
                           for e in range(CB)]
                exp_tiles = {}

                def consume(t, psum_z=psum_z, psum_pv=psum_pv,
                            exp_tiles=exp_tiles):
                    ep = exp_tiles.pop(t)
                    # softmax denominator: M=32 ones weight, all 32 psum rows
                    # hold the same column sum
                    nc.tensor.matmul(psum_z, ones8, ep,
                                     start=(t == 0), stop=(t == JP - 1),
                                     perf_mode=DR)
                    for e in range(CB):
                        nc.tensor.matmul(psum_pv[e],
                                         vT[:, 2 * t:2 * t + 2,
                                            e * P:(e + 1) * P], ep,
                                         start=(t == 0), stop=(t == JP - 1),
                                         perf_mode=DR)

                for t in range(JP):
                    # row-packed pair of K=64 score matmuls (array rows 0-63 /
                    # 64-127) running concurrently on separate psum banks
                    ja, jb = 2 * t, 2 * t + 1
                    psa = mps.tile([P, NB], F32, tag="s", bufs=3, name="psa")
                    psb = mps.tile([P, NB], F32, tag="s", bufs=3, name="psb")
                    nc.tensor.matmul(psa, kT[0:D, ja * P:(ja + 1) * P],
                                     qT[0:D, isl], start=True, stop=True)
                    nc.tensor.matmul(psb, kT[D:2 * D, jb * P:(jb + 1) * P],
                                     qT[D:2 * D, isl], start=True, stop=True,
                                     tile_position=(D, 0))
                    ep = expp.tile([P, 2, NB], F8, tag="exp", name="ep")
                    with nc.allow_low_precision(reason="fp8 attention pipeline"):
                        nc.scalar.activation(ep[:, 0, :], psa,
                                             mybir.ActivationFunctionType.Exp,
                                             bias=expbias, scale=EXPSCALE)
                        nc.scalar.activation(ep[:, 1, :], psb,
                                             mybir.ActivationFunctionType.Exp,
                                             bias=expbias, scale=EXPSCALE)
                    exp_tiles[t] = ep
                    if t >= 2:
                        consume(t - 2)
                consume(JP - 2)
                consume(JP - 1)

                # Z -> gamma/(16*Z), broadcast across partitions via K=1 matmul
                zt = small.tile([1, NB], F32, tag="zt")
                zscale = WSCALE / gamma if gamma != 0 else 1.0
                nc.vector.tensor_scalar_mul(zt, psum_z[0:1, :], zscale)
                rz = small.tile([1, NB], F32, tag="rz")
                nc.vector.reciprocal_approx_fast(rz, zt)
                rzr = small.tile([1, NB], F32R, tag="rzr")
                with nc.allow_low_precision(reason="f32r is 32-bit"):
                    nc.vector.tensor_copy(rzr, rz)
                psrz = mps.tile([P, NB], F32, tag="z")
                nc.tensor.matmul(psrz, gamma_col, rzr,
                                 start=True, stop=True)
                rzb = small.tile([P, NB], F32, tag="rzb")
                nc.vector.tensor_copy(rzb, psrz)

                for e in range(CB):
                    esl = slice(e * P, (e + 1) * P)
                    # mult on DVE straight from PSUM (frees the bank for the
                    # next block); residual adds split across GpSimd and DVE
                    ot = ost.tile([P, NB], F32, tag="o")
                    nc.vector.tensor_tensor(ot, psum_pv[e], rzb,
                                            op=mybir.AluOpType.mult)
                    add_eng = nc.gpsimd if e < 2 else nc.vector
                    add_eng.tensor_tensor(ot, ot, xsb[:, e, isl],
                                          op=mybir.AluOpType.add)
                    nc.sync.dma_start(out=out[esl, isl], in_=ot)

    nc.compile()
    return nc


def kernel(**inputs):
    global LAST_RESULTS
    x = np.asarray(inputs["x"], dtype=np.float32)
    Wq = np.asarray(inputs["Wq"], dtype=np.float32)
    Wk = np.asarray(inputs["Wk"], dtype=np.float32)
    Wv = np.asarray(inputs["Wv"], dtype=np.float32)
    bq = np.asarray(inputs["bq"], dtype=np.float32).reshape(D)
    bk = np.asarray(inputs["bk"], dtype=np.float32).reshape(D)
    bv = np.asarray(inputs["bv"], dtype=np.float32).reshape(C)
    gamma = float(np.asarray(inputs["gamma"]).reshape(-1)[0])

    bv_nonzero = bool(np.any(bv != 0.0))
    nc = build_nc(gamma, bv_nonzero)

    # weight layouts: [p, chunk, m] with source channel c = chunk*128 + p
    wqk = np.concatenate([Wq, Wk], axis=0) * WSCALE        # [128, 512]
    wqk8 = np.ascontiguousarray(
        wqk.reshape(P, CB, P).transpose(2, 1, 0).reshape(P, CB * P)
    ).astype(NP8)
    wv8 = np.ascontiguousarray(
        (Wv * WSCALE).reshape(C, CB, P).transpose(2, 1, 0).reshape(P, CB * C)
    ).astype(NP8)
    bqk16 = (np.concatenate([bq, bk]) * WSCALE).reshape(P, 1)
    gbv = (gamma * bv).reshape(C, 1)

    in_maps = []
    for b in range(B):
        xb = x[b].reshape(C, N)
        x8 = np.ascontiguousarray(
            xb.reshape(CB, P, N).transpose(1, 0, 2).reshape(P, CB * N)
        ).astype(NP8)
        in_maps.append({
            "x8": x8,
            "xf": np.ascontiguousarray(xb),
            "wqk8": wqk8,
            "wv8": wv8,
            "bqk16": bqk16,
            "gbv": gbv,
        })

    res = run_bass_kernel_spmd(nc, in_maps, list(range(B)))
    LAST_RESULTS = res
    out = np.stack([res.results[b]["out"].reshape(C, HW, HW) for b in range(B)])
    return out.astype(np.float32)
